# revision 1
# baseline (speedup 1.0000x reference)
"""Trainium2 Bass kernel for a noisy LSTMCell forward.

  gates = input @ W_ih.T + b_ih + hx @ W_hh.T + b_hh          # [B, 4H]
  i, f, g, o = split(gates); i,f,o=sigmoid, g=tanh
  cy = f*cx + i*g + sqrt(noise_e)*eps_c
  hy = o*tanh(cy) + sqrt(noise_q)*eps_h

B=4096, I=H=1024. Sharding: 2D grid over 8 NeuronCores — 4 batch shards
x 2 gate shards (minimizes per-core HBM traffic vs pure data-parallel:
34MB vs 46MB per core, fp32 end to end).

Everything on device is kept feature-major ([feature, batch]) so that the
matmul contraction dim lands on SBUF partitions with zero on-device
transposes and the ACT engine's per-partition bias applies the gate bias
directly during PSUM eviction. Host-side prep (numpy): concat/transpose
of activations, pre-tiling of W so every DMA is contiguous per partition.
"""

import os
import sys
import numpy as np

for _p in ("/opt/trn_rl_repo", "/root/.axon_site/_ro/trn_rl_repo"):
    if _p not in sys.path and os.path.isdir(_p):
        sys.path.append(_p)

B, I, H = 4096, 1024, 1024
G = 4 * H                 # gate rows total
K = I + H                 # contraction dim
P_B, P_G = 4, 2           # batch shards x gate shards = 8 cores
BS = B // P_B             # 1024 batch cols per core
HS = H // P_G             # 512 h rows per core
NKT = K // 128            # 16 contraction tiles
NHT = HS // 128           # 4 h tiles per core
NA = NHT * 4              # 16 weight blocks (ht-major, gate-minor)
NBC = BS // 512           # 2 batch chunks of 512 (fp32 PSUM free-dim max)

_LAST = None              # BassKernelResults of the most recent run (for test.py)


def _build_nc(mm_dt="float32r", skip_ew=False, wbufs=3):
    import concourse.bacc as bacc
    import concourse.tile as tile
    from concourse import mybir
    from contextlib import ExitStack

    f32 = mybir.dt.float32
    mdt = getattr(mybir.dt, mm_dt)
    AF = mybir.ActivationFunctionType
    nc = bacc.Bacc("TRN2", target_bir_lowering=False)

    xT = nc.declare_dram_parameter("xT", [K, BS], mdt, isOutput=False)
    w = nc.declare_dram_parameter("w", [NA, 128, NKT * 128], mdt, isOutput=False)
    bias = nc.declare_dram_parameter("bias", [128, NA], f32, isOutput=False)
    cxT = nc.declare_dram_parameter("cxT", [HS, BS], f32, isOutput=False)
    epcT = nc.declare_dram_parameter("epcT", [HS, BS], f32, isOutput=False)
    ephT = nc.declare_dram_parameter("ephT", [HS, BS], f32, isOutput=False)
    noise = nc.declare_dram_parameter("noise", [2], f32, isOutput=False)
    hyT = nc.declare_dram_parameter("hyT", [HS, BS], f32, isOutput=True)
    cyT = nc.declare_dram_parameter("cyT", [HS, BS], f32, isOutput=True)

    with tile.TileContext(nc) as tc, ExitStack() as ctx:
        xpool = ctx.enter_context(tc.tile_pool(name="xpool", bufs=1))
        wpool = ctx.enter_context(tc.tile_pool(name="wpool", bufs=wbufs))
        psum = ctx.enter_context(tc.tile_pool(name="psum", bufs=4, space="PSUM"))
        gates = ctx.enter_context(tc.tile_pool(name="gates", bufs=2))
        ew = ctx.enter_context(tc.tile_pool(name="ew", bufs=2))
        const = ctx.enter_context(tc.tile_pool(name="const", bufs=1))

        # Constants: gate biases [128, NA] and sqrt of the two noise scalars
        bias_t = const.tile([128, NA], f32)
        nc.sync.dma_start(out=bias_t[:], in_=bias[:, :])
        se_t = const.tile([128, 1], f32)   # sqrt(noise_e), bcast over partitions
        sq_t = const.tile([128, 1], f32)   # sqrt(noise_q)
        nc.sync.dma_start(out=se_t[:], in_=noise[0:1].to_broadcast([128, 1]))
        nc.sync.dma_start(out=sq_t[:], in_=noise[1:2].to_broadcast([128, 1]))
        nc.scalar.sqrt(se_t[:], se_t[:])
        nc.scalar.sqrt(sq_t[:], sq_t[:])

        # Resident activations: all K tiles of (input;hx)^T for this core's
        # batch shard. [128, BS] each, contiguous per partition in DRAM.
        xk = []
        for kt in range(NKT):
            t = xpool.tile([128, BS], mdt, tag=f"xk{kt}", name=f"xk{kt}")
            nc.sync.dma_start(out=t[:], in_=xT[kt * 128:(kt + 1) * 128, :])
            xk.append(t)

        for ht in range(NHT):
            gt = [gates.tile([128, BS], f32, tag=f"g{gate}", name=f"gt{gate}") for gate in range(4)]
            for gate in range(4):
                a = ht * 4 + gate
                w_t = wpool.tile([128, NKT * 128], mdt)
                nc.sync.dma_start(out=w_t[:], in_=w[a, :, :])
                ps = [psum.tile([128, 512], f32, name=f"ps{c}") for c in range(NBC)]
                for kt in range(NKT):
                    lhsT = w_t[:, kt * 128:(kt + 1) * 128]
                    for c in range(NBC):
                        nc.tensor.matmul(
                            ps[c][:],
                            lhsT,
                            xk[kt][:, c * 512:(c + 1) * 512],
                            start=(kt == 0),
                            stop=(kt == NKT - 1),
                        )
                func = AF.Tanh if gate == 2 else AF.Sigmoid
                for c in range(NBC):
                    nc.scalar.activation(
                        gt[gate][:, c * 512:(c + 1) * 512],
                        ps[c][:],
                        func,
                        bias=bias_t[:, a:a + 1],
                    )

            if skip_ew:
                nc.sync.dma_start(out=cyT[slice(ht*128,(ht+1)*128), :], in_=gt[1][:])
                nc.sync.dma_start(out=hyT[slice(ht*128,(ht+1)*128), :], in_=gt[3][:])
                continue
            # Elementwise combine for this h tile (all [128, BS], in place on
            # the gate tiles: f_t becomes cy, o_t becomes hy).
            i_t, f_t, g_t, o_t = gt
            row = slice(ht * 128, (ht + 1) * 128)
            cx_t = ew.tile([128, BS], f32, tag="cx")
            ec_t = ew.tile([128, BS], f32, tag="ec")
            eh_t = ew.tile([128, BS], f32, tag="eh")
            nc.sync.dma_start(out=cx_t[:], in_=cxT[row, :])
            nc.sync.dma_start(out=ec_t[:], in_=epcT[row, :])
            nc.sync.dma_start(out=eh_t[:], in_=ephT[row, :])

            nc.vector.tensor_mul(f_t[:], f_t[:], cx_t[:])          # f*cx
            nc.vector.tensor_mul(i_t[:], i_t[:], g_t[:])           # i*g
            nc.vector.tensor_add(f_t[:], f_t[:], i_t[:])
            nc.scalar.activation(ec_t[:], ec_t[:], AF.Copy, scale=se_t[:, 0:1])
            nc.vector.tensor_add(f_t[:], f_t[:], ec_t[:])          # = cy
            nc.scalar.activation(g_t[:], f_t[:], AF.Tanh)          # tanh(cy)
            nc.vector.tensor_mul(o_t[:], o_t[:], g_t[:])           # o*tanh(cy)
            nc.scalar.activation(eh_t[:], eh_t[:], AF.Copy, scale=sq_t[:, 0:1])
            nc.vector.tensor_add(o_t[:], o_t[:], eh_t[:])          # = hy
            nc.sync.dma_start(out=cyT[row, :], in_=f_t[:])
            nc.sync.dma_start(out=hyT[row, :], in_=o_t[:])

    nc.compile()
    return nc


def _prep_inputs(input, hx, cx, noise_q, noise_e,
                 weight_ih, weight_hh, bias_ih, bias_hh, eps_c, eps_h):
    f = lambda a: np.ascontiguousarray(np.asarray(a, dtype=np.float32))
    X = np.concatenate([f(input), f(hx)], axis=1)          # [B, K]
    XT = np.ascontiguousarray(X.T)                          # [K, B]
    W_cat = np.concatenate([f(weight_ih), f(weight_hh)], axis=1)   # [G, K]
    bias_full = f(bias_ih) + f(bias_hh)                     # [G]
    cxT = f(cx).T
    epcT = f(eps_c).T
    ephT = f(eps_h).T
    noise = np.array([np.asarray(noise_e).reshape(-1)[0],
                      np.asarray(noise_q).reshape(-1)[0]], dtype=np.float32)

    # Per gate-shard j: weight blocks in the exact consumption order
    # (a = ht*4 + gate), each pre-transposed to [k_p, kt*128 + g_c] so the
    # per-partition DMA stride is a single contiguous 8KB run.
    w_host, bias_host = [], []
    for j in range(P_G):
        blocks, bcols = [], []
        for ht in range(NHT):
            for gate in range(4):
                g0 = gate * H + j * HS + ht * 128
                blk = W_cat[g0:g0 + 128, :]                        # (g_c, k)
                blocks.append(blk.reshape(128, NKT, 128).transpose(2, 1, 0))
                bcols.append(bias_full[g0:g0 + 128])
        w_host.append(np.ascontiguousarray(
            np.stack(blocks).reshape(NA, 128, NKT * 128)))
        bias_host.append(np.ascontiguousarray(np.stack(bcols, axis=1)))

    in_maps = []
    for bi in range(P_B):
        bcol = slice(bi * BS, (bi + 1) * BS)
        for j in range(P_G):
            hrow = slice(j * HS, (j + 1) * HS)
            in_maps.append({
                "xT": np.ascontiguousarray(XT[:, bcol]),
                "w": w_host[j],
                "bias": bias_host[j],
                "cxT": np.ascontiguousarray(cxT[hrow, bcol]),
                "epcT": np.ascontiguousarray(epcT[hrow, bcol]),
                "ephT": np.ascontiguousarray(ephT[hrow, bcol]),
                "noise": noise,
            })
    return in_maps


def _gather(results):
    hyT = np.empty((H, B), dtype=np.float32)
    cyT = np.empty((H, B), dtype=np.float32)
    idx = 0
    for bi in range(P_B):
        bcol = slice(bi * BS, (bi + 1) * BS)
        for j in range(P_G):
            hrow = slice(j * HS, (j + 1) * HS)
            hyT[hrow, bcol] = results[idx]["hyT"]
            cyT[hrow, bcol] = results[idx]["cyT"]
            idx += 1
    return np.ascontiguousarray(hyT.T), np.ascontiguousarray(cyT.T)


def kernel(**inputs):
    global _LAST
    from concourse.bass_utils import run_bass_kernel_spmd

    in_maps = _prep_inputs(**inputs)
    nc = _build_nc()
    _LAST = run_bass_kernel_spmd(nc, in_maps, list(range(8)), trace=False)
    return _gather(_LAST.results)


# ---------------------------------------------------------------------------
# Timing helper for test.py (not used by the grading path): chains n_iter
# data-dependent NEFF executions inside one jit with device-resident inputs;
# the (t_N - t_1)/(N-1) slope is the per-execution hardware time.
# ---------------------------------------------------------------------------

def benchmark(inputs, n_iter=9, reps=5):
    in_maps = _prep_inputs(**inputs)
    nc = _build_nc()
    per_exec_ns, t1_ns, results = _bench_nc(nc, in_maps, n_iter, reps)
    return per_exec_ns, t1_ns, _gather(results)


def _bench_nc(nc, in_maps, n_iter=9, reps=5):
    import time
    import jax
    from jax.sharding import Mesh, PartitionSpec, NamedSharding
    from jax.experimental.shard_map import shard_map
    from concourse import bass2jax, mybir
    from concourse.bass2jax import _bass_exec_p

    bass2jax.install_neuronx_cc_hook()
    assert nc.dbg_addr is None
    partition_name = nc.partition_id_tensor.name if nc.partition_id_tensor else None

    in_names, out_names, out_avals, zero_outs = [], [], [], []
    for alloc in nc.m.functions[0].allocations:
        if not isinstance(alloc, mybir.MemoryLocationSet):
            continue
        name = alloc.memorylocations[0].name
        if alloc.kind == "ExternalInput":
            if name != partition_name:
                in_names.append(name)
        elif alloc.kind == "ExternalOutput":
            shape = tuple(alloc.tensor_shape)
            dtype = mybir.dt.np(alloc.dtype)
            out_names.append(name)
            out_avals.append(jax.core.ShapedArray(shape, dtype))
            zero_outs.append(np.zeros(shape, dtype))
    n_params = len(in_names)
    all_in_names = tuple(in_names + out_names
                         + ([partition_name] if partition_name else []))

    def make_body(iters):
        def _body(*args):
            ins = list(args[:n_params])
            outs = tuple(args[n_params:])

            def one(outs):
                pid = [bass2jax.partition_id_tensor()] if partition_name else []
                return tuple(_bass_exec_p.bind(
                    *ins, *outs, *pid,
                    out_avals=tuple(out_avals),
                    in_names=all_in_names,
                    out_names=tuple(out_names),
                    lowering_input_output_aliases=(),
                    sim_require_finite=True,
                    sim_require_nnan=True,
                    nc=nc,
                ))

            return one(outs)
        return _body

    n_cores = 8
    devices = jax.devices()[:n_cores]
    mesh = Mesh(np.asarray(devices), ("core",))
    spec = NamedSharding(mesh, PartitionSpec("core"))
    in_specs = (PartitionSpec("core"),) * (n_params + len(out_names))
    out_specs = (PartitionSpec("core"),) * len(out_names)

    concat_in = [
        np.concatenate([np.asarray(in_maps[c][name]) for c in range(n_cores)], axis=0)
        for name in in_names
    ]
    concat_zeros = [
        np.zeros((n_cores * z.shape[0], *z.shape[1:]), z.dtype) for z in zero_outs
    ]
    dev_args = [jax.device_put(a, spec) for a in concat_in + concat_zeros]
    jax.block_until_ready(dev_args)

    fn = jax.jit(shard_map(make_body(1), mesh=mesh, in_specs=in_specs,
                           out_specs=out_specs, check_rep=False),
                 keep_unused=True)
    out1 = fn(*dev_args)          # compile + warm
    jax.block_until_ready(out1)

    def timed(iters):
        best = float("inf")
        for _ in range(reps):
            t0 = time.perf_counter()
            out = None
            for _i in range(iters):
                out = fn(*dev_args)   # async dispatches queue in order
            jax.block_until_ready(out)
            best = min(best, time.perf_counter() - t0)
        return best

    t1 = timed(1)
    tn = timed(n_iter)
    per_exec_ns = (tn - t1) / (n_iter - 1) * 1e9
    results = [
        {name: np.asarray(out1[i]).reshape(n_cores, *out_avals[i].shape)[c]
         for i, name in enumerate(out_names)}
        for c in range(n_cores)
    ]
    return per_exec_ns, t1 * 1e9, results



# revision 3
# speedup vs baseline: 4.0151x; 4.0151x over previous
"""Trainium2 Bass kernel for a noisy LSTMCell forward.

  gates = input @ W_ih.T + b_ih + hx @ W_hh.T + b_hh          # [B, 4H]
  i, f, g, o = split(gates); i,f,o=sigmoid, g=tanh
  cy = f*cx + i*g + sqrt(noise_e)*eps_c
  hy = o*tanh(cy) + sqrt(noise_q)*eps_h

B=4096, I=H=1024. Sharding: 2D grid over 8 NeuronCores — 4 batch shards
x 2 gate shards (minimizes per-core HBM traffic vs pure data-parallel:
34MB vs 46MB per core, fp32 end to end).

Everything on device is kept feature-major ([feature, batch]) so that the
matmul contraction dim lands on SBUF partitions with zero on-device
transposes and the ACT engine's per-partition bias applies the gate bias
directly during PSUM eviction. Host-side prep (numpy): concat/transpose
of activations, pre-tiling of W so every DMA is contiguous per partition.
"""

import os
import sys
import numpy as np

for _p in ("/opt/trn_rl_repo", "/root/.axon_site/_ro/trn_rl_repo"):
    if _p not in sys.path and os.path.isdir(_p):
        sys.path.append(_p)

B, I, H = 4096, 1024, 1024
G = 4 * H                 # gate rows total
K = I + H                 # contraction dim
P_B, P_G = 4, 2           # batch shards x gate shards = 8 cores
BS = B // P_B             # 1024 batch cols per core
HS = H // P_G             # 512 h rows per core
NKT = K // 128            # 16 contraction tiles
NHT = HS // 128           # 4 h tiles per core
NA = NHT * 4              # 16 weight blocks (ht-major, gate-minor)
NBC = BS // 512           # 2 batch chunks of 512 (fp32 PSUM free-dim max)

_LAST = None              # BassKernelResults of the most recent run (for test.py)


def _build_nc(mm_dt="float16", skip_ew=False, wbufs=3):
    import concourse.bacc as bacc
    import concourse.tile as tile
    from concourse import mybir
    from contextlib import ExitStack

    f32 = mybir.dt.float32
    mdt = getattr(mybir.dt, mm_dt)
    AF = mybir.ActivationFunctionType
    nc = bacc.Bacc("TRN2", target_bir_lowering=False)

    xT = nc.declare_dram_parameter("xT", [K, BS], mdt, isOutput=False)
    w = nc.declare_dram_parameter("w", [NA, 128, NKT * 128], mdt, isOutput=False)
    bias = nc.declare_dram_parameter("bias", [128, NA], f32, isOutput=False)
    cxT = nc.declare_dram_parameter("cxT", [HS, BS], f32, isOutput=False)
    epcT = nc.declare_dram_parameter("epcT", [HS, BS], f32, isOutput=False)
    ephT = nc.declare_dram_parameter("ephT", [HS, BS], f32, isOutput=False)
    noise = nc.declare_dram_parameter("noise", [2], f32, isOutput=False)
    hyT = nc.declare_dram_parameter("hyT", [HS, BS], f32, isOutput=True)
    cyT = nc.declare_dram_parameter("cyT", [HS, BS], f32, isOutput=True)

    with tile.TileContext(nc) as tc, ExitStack() as ctx:
        xpool = ctx.enter_context(tc.tile_pool(name="xpool", bufs=1))
        wpool = ctx.enter_context(tc.tile_pool(name="wpool", bufs=wbufs))
        psum = ctx.enter_context(tc.tile_pool(name="psum", bufs=4, space="PSUM"))
        gates = ctx.enter_context(tc.tile_pool(name="gates", bufs=2))
        ew = ctx.enter_context(tc.tile_pool(name="ew", bufs=2))
        const = ctx.enter_context(tc.tile_pool(name="const", bufs=1))

        # Constants: gate biases [128, NA] and sqrt of the two noise scalars
        bias_t = const.tile([128, NA], f32)
        nc.sync.dma_start(out=bias_t[:], in_=bias[:, :])
        se_t = const.tile([128, 1], f32)   # sqrt(noise_e), bcast over partitions
        sq_t = const.tile([128, 1], f32)   # sqrt(noise_q)
        nc.sync.dma_start(out=se_t[:], in_=noise[0:1].to_broadcast([128, 1]))
        nc.sync.dma_start(out=sq_t[:], in_=noise[1:2].to_broadcast([128, 1]))
        nc.scalar.sqrt(se_t[:], se_t[:])
        nc.scalar.sqrt(sq_t[:], sq_t[:])

        # Resident activations: all K tiles of (input;hx)^T for this core's
        # batch shard. [128, BS] each, contiguous per partition in DRAM.
        xk = []
        for kt in range(NKT):
            t = xpool.tile([128, BS], mdt, tag=f"xk{kt}", name=f"xk{kt}")
            nc.sync.dma_start(out=t[:], in_=xT[kt * 128:(kt + 1) * 128, :])
            xk.append(t)

        for ht in range(NHT):
            gt = [gates.tile([128, BS], f32, tag=f"g{gate}", name=f"gt{gate}") for gate in range(4)]
            for gate in range(4):
                a = ht * 4 + gate
                w_t = wpool.tile([128, NKT * 128], mdt)
                nc.sync.dma_start(out=w_t[:], in_=w[a, :, :])
                ps = [psum.tile([128, 512], f32, name=f"ps{c}") for c in range(NBC)]
                for kt in range(NKT):
                    lhsT = w_t[:, kt * 128:(kt + 1) * 128]
                    for c in range(NBC):
                        nc.tensor.matmul(
                            ps[c][:],
                            lhsT,
                            xk[kt][:, c * 512:(c + 1) * 512],
                            start=(kt == 0),
                            stop=(kt == NKT - 1),
                        )
                func = AF.Tanh if gate == 2 else AF.Sigmoid
                for c in range(NBC):
                    nc.scalar.activation(
                        gt[gate][:, c * 512:(c + 1) * 512],
                        ps[c][:],
                        func,
                        bias=bias_t[:, a:a + 1],
                    )

            if skip_ew:
                nc.sync.dma_start(out=cyT[slice(ht*128,(ht+1)*128), :], in_=gt[1][:])
                nc.sync.dma_start(out=hyT[slice(ht*128,(ht+1)*128), :], in_=gt[3][:])
                continue
            # Elementwise combine for this h tile (all [128, BS], in place on
            # the gate tiles: f_t becomes cy, o_t becomes hy).
            i_t, f_t, g_t, o_t = gt
            row = slice(ht * 128, (ht + 1) * 128)
            cx_t = ew.tile([128, BS], f32, tag="cx")
            ec_t = ew.tile([128, BS], f32, tag="ec")
            eh_t = ew.tile([128, BS], f32, tag="eh")
            nc.sync.dma_start(out=cx_t[:], in_=cxT[row, :])
            nc.sync.dma_start(out=ec_t[:], in_=epcT[row, :])
            nc.sync.dma_start(out=eh_t[:], in_=ephT[row, :])

            nc.vector.tensor_mul(f_t[:], f_t[:], cx_t[:])          # f*cx
            nc.vector.tensor_mul(i_t[:], i_t[:], g_t[:])           # i*g
            nc.vector.tensor_add(f_t[:], f_t[:], i_t[:])
            nc.scalar.activation(ec_t[:], ec_t[:], AF.Copy, scale=se_t[:, 0:1])
            nc.vector.tensor_add(f_t[:], f_t[:], ec_t[:])          # = cy
            nc.scalar.activation(g_t[:], f_t[:], AF.Tanh)          # tanh(cy)
            nc.vector.tensor_mul(o_t[:], o_t[:], g_t[:])           # o*tanh(cy)
            nc.scalar.activation(eh_t[:], eh_t[:], AF.Copy, scale=sq_t[:, 0:1])
            nc.vector.tensor_add(o_t[:], o_t[:], eh_t[:])          # = hy
            nc.sync.dma_start(out=cyT[row, :], in_=f_t[:])
            nc.sync.dma_start(out=hyT[row, :], in_=o_t[:])

    nc.compile()
    return nc


def _prep_inputs(input, hx, cx, noise_q, noise_e,
                 weight_ih, weight_hh, bias_ih, bias_hh, eps_c, eps_h,
                 mm_np=np.float16):
    f = lambda a: np.ascontiguousarray(np.asarray(a, dtype=np.float32))
    X = np.concatenate([f(input), f(hx)], axis=1)          # [B, K]
    XT = np.ascontiguousarray(X.T.astype(mm_np))            # [K, B]
    W_cat = np.concatenate([f(weight_ih), f(weight_hh)], axis=1).astype(mm_np)  # [G, K]
    bias_full = f(bias_ih) + f(bias_hh)                     # [G]
    cxT = f(cx).T
    epcT = f(eps_c).T
    ephT = f(eps_h).T
    noise = np.array([np.asarray(noise_e).reshape(-1)[0],
                      np.asarray(noise_q).reshape(-1)[0]], dtype=np.float32)

    # Per gate-shard j: weight blocks in the exact consumption order
    # (a = ht*4 + gate), each pre-transposed to [k_p, kt*128 + g_c] so the
    # per-partition DMA stride is a single contiguous 8KB run.
    w_host, bias_host = [], []
    for j in range(P_G):
        blocks, bcols = [], []
        for ht in range(NHT):
            for gate in range(4):
                g0 = gate * H + j * HS + ht * 128
                blk = W_cat[g0:g0 + 128, :]                        # (g_c, k)
                blocks.append(blk.reshape(128, NKT, 128).transpose(2, 1, 0))
                bcols.append(bias_full[g0:g0 + 128])
        w_host.append(np.ascontiguousarray(
            np.stack(blocks).reshape(NA, 128, NKT * 128)))
        bias_host.append(np.ascontiguousarray(np.stack(bcols, axis=1)))

    in_maps = []
    for bi in range(P_B):
        bcol = slice(bi * BS, (bi + 1) * BS)
        for j in range(P_G):
            hrow = slice(j * HS, (j + 1) * HS)
            in_maps.append({
                "xT": np.ascontiguousarray(XT[:, bcol]),
                "w": w_host[j],
                "bias": bias_host[j],
                "cxT": np.ascontiguousarray(cxT[hrow, bcol]),
                "epcT": np.ascontiguousarray(epcT[hrow, bcol]),
                "ephT": np.ascontiguousarray(ephT[hrow, bcol]),
                "noise": noise,
            })
    return in_maps


def _gather(results):
    hyT = np.empty((H, B), dtype=np.float32)
    cyT = np.empty((H, B), dtype=np.float32)
    idx = 0
    for bi in range(P_B):
        bcol = slice(bi * BS, (bi + 1) * BS)
        for j in range(P_G):
            hrow = slice(j * HS, (j + 1) * HS)
            hyT[hrow, bcol] = results[idx]["hyT"]
            cyT[hrow, bcol] = results[idx]["cyT"]
            idx += 1
    return np.ascontiguousarray(hyT.T), np.ascontiguousarray(cyT.T)


def kernel(**inputs):
    global _LAST
    from concourse.bass_utils import run_bass_kernel_spmd

    in_maps = _prep_inputs(**inputs)
    nc = _build_nc()
    _LAST = run_bass_kernel_spmd(nc, in_maps, list(range(8)), trace=False)
    return _gather(_LAST.results)


# ---------------------------------------------------------------------------
# Timing helper for test.py (not used by the grading path): chains n_iter
# data-dependent NEFF executions inside one jit with device-resident inputs;
# the (t_N - t_1)/(N-1) slope is the per-execution hardware time.
# ---------------------------------------------------------------------------

def benchmark(inputs, n_iter=9, reps=5):
    in_maps = _prep_inputs(**inputs)
    nc = _build_nc()
    per_exec_ns, t1_ns, results = _bench_nc(nc, in_maps, n_iter, reps)
    return per_exec_ns, t1_ns, _gather(results)


def _bench_nc(nc, in_maps, n_iter=9, reps=5):
    import time
    import jax
    from jax.sharding import Mesh, PartitionSpec, NamedSharding
    from jax.experimental.shard_map import shard_map
    from concourse import bass2jax, mybir
    from concourse.bass2jax import _bass_exec_p

    bass2jax.install_neuronx_cc_hook()
    assert nc.dbg_addr is None
    partition_name = nc.partition_id_tensor.name if nc.partition_id_tensor else None

    in_names, out_names, out_avals, zero_outs = [], [], [], []
    for alloc in nc.m.functions[0].allocations:
        if not isinstance(alloc, mybir.MemoryLocationSet):
            continue
        name = alloc.memorylocations[0].name
        if alloc.kind == "ExternalInput":
            if name != partition_name:
                in_names.append(name)
        elif alloc.kind == "ExternalOutput":
            shape = tuple(alloc.tensor_shape)
            dtype = mybir.dt.np(alloc.dtype)
            out_names.append(name)
            out_avals.append(jax.core.ShapedArray(shape, dtype))
            zero_outs.append(np.zeros(shape, dtype))
    n_params = len(in_names)
    all_in_names = tuple(in_names + out_names
                         + ([partition_name] if partition_name else []))

    def make_body(iters):
        def _body(*args):
            ins = list(args[:n_params])
            outs = tuple(args[n_params:])

            def one(outs):
                pid = [bass2jax.partition_id_tensor()] if partition_name else []
                return tuple(_bass_exec_p.bind(
                    *ins, *outs, *pid,
                    out_avals=tuple(out_avals),
                    in_names=all_in_names,
                    out_names=tuple(out_names),
                    lowering_input_output_aliases=(),
                    sim_require_finite=True,
                    sim_require_nnan=True,
                    nc=nc,
                ))

            return one(outs)
        return _body

    n_cores = 8
    devices = jax.devices()[:n_cores]
    mesh = Mesh(np.asarray(devices), ("core",))
    spec = NamedSharding(mesh, PartitionSpec("core"))
    in_specs = (PartitionSpec("core"),) * (n_params + len(out_names))
    out_specs = (PartitionSpec("core"),) * len(out_names)

    concat_in = [
        np.concatenate([np.asarray(in_maps[c][name]) for c in range(n_cores)], axis=0)
        for name in in_names
    ]
    concat_zeros = [
        np.zeros((n_cores * z.shape[0], *z.shape[1:]), z.dtype) for z in zero_outs
    ]
    dev_args = [jax.device_put(a, spec) for a in concat_in + concat_zeros]
    jax.block_until_ready(dev_args)

    fn = jax.jit(shard_map(make_body(1), mesh=mesh, in_specs=in_specs,
                           out_specs=out_specs, check_rep=False),
                 keep_unused=True)
    out1 = fn(*dev_args)          # compile + warm
    jax.block_until_ready(out1)

    def timed(iters):
        best = float("inf")
        for _ in range(reps):
            t0 = time.perf_counter()
            out = None
            for _i in range(iters):
                out = fn(*dev_args)   # async dispatches queue in order
            jax.block_until_ready(out)
            best = min(best, time.perf_counter() - t0)
        return best

    t1 = timed(1)
    tn = timed(n_iter)
    per_exec_ns = (tn - t1) / (n_iter - 1) * 1e9
    results = [
        {name: np.asarray(out1[i]).reshape(n_cores, *out_avals[i].shape)[c]
         for i, name in enumerate(out_names)}
        for c in range(n_cores)
    ]
    return per_exec_ns, t1 * 1e9, results



# revision 8
# speedup vs baseline: 35.1939x; 8.7653x over previous
"""Trainium2 Bass kernel for a noisy LSTMCell forward.

  gates = input @ W_ih.T + b_ih + hx @ W_hh.T + b_hh          # [B, 4H]
  i, f, g, o = split(gates); i,f,o=sigmoid, g=tanh
  cy = f*cx + i*g + sqrt(noise_e)*eps_c
  hy = o*tanh(cy) + sqrt(noise_q)*eps_h

B=4096, I=H=1024. Sharding: 2D grid over 8 NeuronCores — 4 batch shards
x 2 gate shards. Matmul inputs are fp16 (PE runs fp16 at full bf16 rate,
4x the fp32 rate; rel err ~8e-3 vs the 2e-2 gate), elementwise tensors and
outputs are fp16 as well (noise scaling folded in on the host).

Everything on device is feature-major ([feature, batch]) so the matmul
contraction lands on SBUF partitions with no on-device transposes, and the
ACT engine applies the per-gate-row bias during PSUM eviction. DMA traffic
is split across the two HWDGE rings (sync + scalar) and the SWDGE ring
(gpsimd) so the weight-block loads are not serialized behind the resident
activation loads. The per-h-tile loop is chunk-major (batch chunks of 512)
so the elementwise tail only covers one chunk.
"""

import os
import sys
import numpy as np

for _p in ("/opt/trn_rl_repo", "/root/.axon_site/_ro/trn_rl_repo"):
    if _p not in sys.path and os.path.isdir(_p):
        sys.path.append(_p)

B, I, H = 4096, 1024, 1024
G = 4 * H                 # gate rows total
K = I + H                 # contraction dim
P_B, P_G = 4, 2           # batch shards x gate shards = 8 cores
BS = B // P_B             # 1024 batch cols per core
HS = H // P_G             # 512 h rows per core
NKT = K // 128            # 16 contraction tiles
NHT = HS // 128           # 4 h tiles per core
NA = NHT * 4              # 16 weight blocks (ht-major, gate-minor)
NBC = BS // 512           # 2 batch chunks of 512 (fp32 PSUM free-dim max)

_LAST = None              # BassKernelResults of the most recent run (for test.py)


def _build_nc(mm_dt="float16", n_rep=1):
    import concourse.bacc as bacc
    import concourse.tile as tile
    from concourse import mybir
    from contextlib import ExitStack

    f32 = mybir.dt.float32
    f16 = mybir.dt.float16
    mdt = getattr(mybir.dt, mm_dt)
    AF = mybir.ActivationFunctionType
    nc = bacc.Bacc("TRN2", target_bir_lowering=False)

    xT = nc.declare_dram_parameter("xT", [K, BS], mdt, isOutput=False)
    w = nc.declare_dram_parameter("w", [NA, 128, NKT * 128], mdt, isOutput=False)
    bias = nc.declare_dram_parameter("bias", [128, NA], f32, isOutput=False)
    cxT = nc.declare_dram_parameter("cxT", [HS, BS], f16, isOutput=False)
    epcT = nc.declare_dram_parameter("epcT", [HS, BS], f16, isOutput=False)
    ephT = nc.declare_dram_parameter("ephT", [HS, BS], f16, isOutput=False)
    hyT = nc.declare_dram_parameter("hyT", [HS, BS], f16, isOutput=True)
    cyT = nc.declare_dram_parameter("cyT", [HS, BS], f16, isOutput=True)

    with tile.TileContext(nc) as tc, ExitStack() as ctx:
        xpool = ctx.enter_context(tc.tile_pool(name="xpool", bufs=1))
        wpool = ctx.enter_context(tc.tile_pool(name="wpool", bufs=2))
        psum = ctx.enter_context(tc.tile_pool(name="psum", bufs=2, space="PSUM"))
        gates = ctx.enter_context(tc.tile_pool(name="gates", bufs=2))
        ew = ctx.enter_context(tc.tile_pool(name="ew", bufs=2))
        const = ctx.enter_context(tc.tile_pool(name="const", bufs=1))

        bias_t = const.tile([128, NA], f32)
        nc.sync.dma_start(out=bias_t[:], in_=bias[:, :])

        for _rep in range(n_rep):
            _body(nc, mdt, f16, f32, AF,
                  xpool, wpool, psum, gates, ew,
                  xT, w, cxT, epcT, ephT, hyT, cyT, bias_t)

    nc.compile()
    return nc


def _body(nc, mdt, f16, f32, AF, xpool, wpool, psum, gates, ew,
          xT, w, cxT, epcT, ephT, hyT, cyT, bias_t):
    # Resident activations: all K tiles of (input;hx)^T for this core's batch
    # shard, [128, BS] each, split across the two HWDGE rings so they land at
    # ~2x the single-ring rate during the lead-in.
    xk = []
    for kt in range(NKT):
        t = xpool.tile([128, BS], mdt, tag=f"xk{kt}", name=f"xk{kt}")
        eng = nc.sync if kt % 2 == 0 else nc.scalar
        eng.dma_start(out=t[:], in_=xT[kt * 128:(kt + 1) * 128, :])
        xk.append(t)

    for ht in range(NHT):
        row = slice(ht * 128, (ht + 1) * 128)
        # Weight blocks for this h tile go on the SWDGE (gpsimd) ring so the
        # first block is not queued behind the 4MB of xk tiles.
        w_t = []
        for gate in range(4):
            a = ht * 4 + gate
            t = wpool.tile([128, NKT * 128], mdt, tag=f"w{gate}", name=f"w{ht}_{gate}")
            nc.gpsimd.dma_start(out=t[:], in_=w[a, :, :])
            w_t.append(t)
        gt = [gates.tile([128, BS], f16, tag=f"g{gate}", name=f"gt{ht}_{gate}")
              for gate in range(4)]
        cx_t = ew.tile([128, BS], f16, tag="cx")
        ec_t = ew.tile([128, BS], f16, tag="ec")
        eh_t = ew.tile([128, BS], f16, tag="eh")
        nc.gpsimd.dma_start(out=cx_t[:], in_=cxT[row, :])
        nc.gpsimd.dma_start(out=ec_t[:], in_=epcT[row, :])
        nc.gpsimd.dma_start(out=eh_t[:], in_=ephT[row, :])

        for c in range(NBC):
            sl = slice(c * 512, (c + 1) * 512)
            for gate in range(4):
                a = ht * 4 + gate
                ps = psum.tile([128, 512], f32, tag=f"ps{gate}", name=f"ps{ht}_{c}_{gate}")
                for kt in range(NKT):
                    nc.tensor.matmul(
                        ps[:],
                        w_t[gate][:, kt * 128:(kt + 1) * 128],
                        xk[kt][:, sl],
                        start=(kt == 0),
                        stop=(kt == NKT - 1),
                    )
                func = AF.Tanh if gate == 2 else AF.Sigmoid
                nc.scalar.activation(gt[gate][:, sl], ps[:], func,
                                     bias=bias_t[:, a:a + 1])

            # Elementwise for this chunk (all fp16; noise pre-scaled on host).
            i_t, f_t, g_t, o_t = (gt[0][:, sl], gt[1][:, sl],
                                  gt[2][:, sl], gt[3][:, sl])
            nc.vector.tensor_mul(f_t, f_t, cx_t[:, sl])        # f*cx
            nc.vector.tensor_mul(i_t, i_t, g_t)                # i*g
            nc.vector.tensor_add(f_t, f_t, i_t)
            nc.vector.tensor_add(f_t, f_t, ec_t[:, sl])        # = cy
            nc.scalar.activation(g_t, f_t, AF.Tanh)            # tanh(cy)
            nc.vector.tensor_mul(o_t, o_t, g_t)                # o*tanh(cy)
            nc.vector.tensor_add(o_t, o_t, eh_t[:, sl])        # = hy
            nc.sync.dma_start(out=cyT[row, sl], in_=f_t)
            nc.scalar.dma_start(out=hyT[row, sl], in_=o_t)


def _prep_inputs(input, hx, cx, noise_q, noise_e,
                 weight_ih, weight_hh, bias_ih, bias_hh, eps_c, eps_h,
                 mm_np=np.float16):
    f = lambda a: np.ascontiguousarray(np.asarray(a, dtype=np.float32))
    X = np.concatenate([f(input), f(hx)], axis=1)          # [B, K]
    XT = np.ascontiguousarray(X.T.astype(mm_np))            # [K, B]
    W_cat = np.concatenate([f(weight_ih), f(weight_hh)], axis=1).astype(mm_np)  # [G, K]
    bias_full = f(bias_ih) + f(bias_hh)                     # [G]
    cxT = f(cx).T.astype(np.float16)
    epcT = (np.sqrt(np.asarray(noise_e, dtype=np.float32).reshape(-1)[0])
            * f(eps_c)).T.astype(np.float16)
    ephT = (np.sqrt(np.asarray(noise_q, dtype=np.float32).reshape(-1)[0])
            * f(eps_h)).T.astype(np.float16)

    # Per gate-shard j: weight blocks in the exact consumption order
    # (a = ht*4 + gate), each pre-transposed to [k_p, kt*128 + g_c] so the
    # per-partition DMA stride is a single contiguous run.
    w_host, bias_host = [], []
    for j in range(P_G):
        blocks, bcols = [], []
        for ht in range(NHT):
            for gate in range(4):
                g0 = gate * H + j * HS + ht * 128
                blk = W_cat[g0:g0 + 128, :]                        # (g_c, k)
                blocks.append(blk.reshape(128, NKT, 128).transpose(2, 1, 0))
                bcols.append(bias_full[g0:g0 + 128])
        w_host.append(np.ascontiguousarray(
            np.stack(blocks).reshape(NA, 128, NKT * 128)))
        bias_host.append(np.ascontiguousarray(np.stack(bcols, axis=1)))

    in_maps = []
    for bi in range(P_B):
        bcol = slice(bi * BS, (bi + 1) * BS)
        for j in range(P_G):
            hrow = slice(j * HS, (j + 1) * HS)
            in_maps.append({
                "xT": np.ascontiguousarray(XT[:, bcol]),
                "w": w_host[j],
                "bias": bias_host[j],
                "cxT": np.ascontiguousarray(cxT[hrow, bcol]),
                "epcT": np.ascontiguousarray(epcT[hrow, bcol]),
                "ephT": np.ascontiguousarray(ephT[hrow, bcol]),
            })
    return in_maps


def _gather(results):
    hyT = np.empty((H, B), dtype=np.float32)
    cyT = np.empty((H, B), dtype=np.float32)
    idx = 0
    for bi in range(P_B):
        bcol = slice(bi * BS, (bi + 1) * BS)
        for j in range(P_G):
            hrow = slice(j * HS, (j + 1) * HS)
            hyT[hrow, bcol] = results[idx]["hyT"].astype(np.float32)
            cyT[hrow, bcol] = results[idx]["cyT"].astype(np.float32)
            idx += 1
    return np.ascontiguousarray(hyT.T), np.ascontiguousarray(cyT.T)


def kernel(**inputs):
    global _LAST
    from concourse.bass_utils import run_bass_kernel_spmd

    in_maps = _prep_inputs(**inputs)
    nc = _build_nc()
    _LAST = run_bass_kernel_spmd(nc, in_maps, list(range(8)), trace=False)
    return _gather(_LAST.results)


# ---------------------------------------------------------------------------
# Timing helper for test.py (not used by the grading path): chains n_iter
# data-dependent NEFF executions inside one jit with device-resident inputs;
# the (t_N - t_1)/(N-1) slope is the per-execution hardware time.
# ---------------------------------------------------------------------------

def benchmark(inputs, n_iter=9, reps=5, n_rep=5):
    in_maps = _prep_inputs(**inputs)
    nc = _build_nc(n_rep=n_rep)
    per_exec_ns, t1_ns, results = _bench_nc(nc, in_maps, n_iter, reps)
    return per_exec_ns / n_rep, t1_ns, _gather(results)


def _bench_nc(nc, in_maps, n_iter=9, reps=5):
    import time
    import jax
    from jax.sharding import Mesh, PartitionSpec, NamedSharding
    from jax.experimental.shard_map import shard_map
    from concourse import bass2jax, mybir
    from concourse.bass2jax import _bass_exec_p

    bass2jax.install_neuronx_cc_hook()
    assert nc.dbg_addr is None
    partition_name = nc.partition_id_tensor.name if nc.partition_id_tensor else None

    in_names, out_names, out_avals, zero_outs = [], [], [], []
    for alloc in nc.m.functions[0].allocations:
        if not isinstance(alloc, mybir.MemoryLocationSet):
            continue
        name = alloc.memorylocations[0].name
        if alloc.kind == "ExternalInput":
            if name != partition_name:
                in_names.append(name)
        elif alloc.kind == "ExternalOutput":
            shape = tuple(alloc.tensor_shape)
            dtype = mybir.dt.np(alloc.dtype)
            out_names.append(name)
            out_avals.append(jax.core.ShapedArray(shape, dtype))
            zero_outs.append(np.zeros(shape, dtype))
    n_params = len(in_names)
    all_in_names = tuple(in_names + out_names
                         + ([partition_name] if partition_name else []))

    def make_body(iters):
        def _body(*args):
            ins = list(args[:n_params])
            outs = tuple(args[n_params:])

            def one(outs):
                pid = [bass2jax.partition_id_tensor()] if partition_name else []
                return tuple(_bass_exec_p.bind(
                    *ins, *outs, *pid,
                    out_avals=tuple(out_avals),
                    in_names=all_in_names,
                    out_names=tuple(out_names),
                    lowering_input_output_aliases=(),
                    sim_require_finite=True,
                    sim_require_nnan=True,
                    nc=nc,
                ))

            return one(outs)
        return _body

    n_cores = 8
    devices = jax.devices()[:n_cores]
    mesh = Mesh(np.asarray(devices), ("core",))
    spec = NamedSharding(mesh, PartitionSpec("core"))
    in_specs = (PartitionSpec("core"),) * (n_params + len(out_names))
    out_specs = (PartitionSpec("core"),) * len(out_names)

    concat_in = [
        np.concatenate([np.asarray(in_maps[c][name]) for c in range(n_cores)], axis=0)
        for name in in_names
    ]
    concat_zeros = [
        np.zeros((n_cores * z.shape[0], *z.shape[1:]), z.dtype) for z in zero_outs
    ]
    dev_args = [jax.device_put(a, spec) for a in concat_in + concat_zeros]
    jax.block_until_ready(dev_args)

    fn = jax.jit(shard_map(make_body(1), mesh=mesh, in_specs=in_specs,
                           out_specs=out_specs, check_rep=False),
                 keep_unused=True)
    out1 = fn(*dev_args)          # compile + warm
    jax.block_until_ready(out1)

    def timed(iters):
        best = float("inf")
        for _ in range(reps):
            t0 = time.perf_counter()
            out = None
            for _i in range(iters):
                out = fn(*dev_args)   # async dispatches queue in order
            jax.block_until_ready(out)
            best = min(best, time.perf_counter() - t0)
        return best

    t1 = timed(1)
    tn = timed(n_iter)
    per_exec_ns = (tn - t1) / (n_iter - 1) * 1e9
    results = [
        {name: np.asarray(out1[i]).reshape(n_cores, *out_avals[i].shape)[c]
         for i, name in enumerate(out_names)}
        for c in range(n_cores)
    ]
    return per_exec_ns, t1 * 1e9, results


# revision 19
# speedup vs baseline: 941.8068x; 26.7605x over previous
"""Trainium2 Bass kernel for a noisy LSTMCell forward.

  gates = input @ W_ih.T + b_ih + hx @ W_hh.T + b_hh          # [B, 4H]
  i, f, g, o = split(gates); i,f,o=sigmoid, g=tanh
  cy = f*cx + i*g + sqrt(noise_e)*eps_c
  hy = o*tanh(cy) + sqrt(noise_q)*eps_h

B=4096, I=H=1024. Sharding: 2D grid over 8 NeuronCores — 4 batch shards
x 2 gate shards. Matmul inputs are fp16 (PE runs fp16 at full bf16 rate,
4x the fp32 rate; rel err ~8e-3 vs the 2e-2 gate), elementwise tensors and
outputs are fp16 as well (noise scaling folded in on the host).

Everything on device is feature-major ([feature, batch]) so the matmul
contraction lands on SBUF partitions with no on-device transposes, and the
ACT engine applies the per-gate-row bias during PSUM eviction. DMA traffic
is split across the two HWDGE rings (sync + scalar) and the SWDGE ring
(gpsimd) so the weight-block loads are not serialized behind the resident
activation loads. The per-h-tile loop is chunk-major (batch chunks of 512)
so the elementwise tail only covers one chunk.
"""

import os
import sys
import numpy as np

for _p in ("/opt/trn_rl_repo", "/root/.axon_site/_ro/trn_rl_repo"):
    if _p not in sys.path and os.path.isdir(_p):
        sys.path.append(_p)

B, I, H = 4096, 1024, 1024
G = 4 * H                 # gate rows total
K = I + H                 # contraction dim
P_B, P_G = 4, 2           # batch shards x gate shards = 8 cores
BS = B // P_B             # 1024 batch cols per core
HS = H // P_G             # 512 h rows per core
NKT = K // 128            # 16 contraction tiles
NHT = HS // 128           # 4 h tiles per core
NA = NHT * 4              # 16 weight blocks (ht-major, gate-minor)
NBC = BS // 512           # 2 batch chunks of 512 (fp32 PSUM free-dim max)

_LAST = None              # BassKernelResults of the most recent run (for test.py)


def _build_nc(mm_dt="float16", n_rep=1):
    import concourse.bacc as bacc
    import concourse.tile as tile
    from concourse import mybir
    from contextlib import ExitStack

    f32 = mybir.dt.float32
    f16 = mybir.dt.float16
    mdt = getattr(mybir.dt, mm_dt)
    AF = mybir.ActivationFunctionType
    nc = bacc.Bacc("TRN2", target_bir_lowering=False)

    xT = nc.declare_dram_parameter("xT", [K, BS], mdt, isOutput=False)
    w = nc.declare_dram_parameter("w", [NA, 128, NKT * 128], mdt, isOutput=False)
    bias = nc.declare_dram_parameter("bias", [128, NA], f32, isOutput=False)
    cxT = nc.declare_dram_parameter("cxT", [128, NHT * BS], f16, isOutput=False)
    epcT = nc.declare_dram_parameter("epcT", [128, NHT * BS], f16, isOutput=False)
    ephT = nc.declare_dram_parameter("ephT", [128, NHT * BS], f16, isOutput=False)
    hyT = nc.declare_dram_parameter("hyT", [HS, BS], f16, isOutput=True)
    cyT = nc.declare_dram_parameter("cyT", [HS, BS], f16, isOutput=True)

    with tile.TileContext(nc) as tc, ExitStack() as ctx:
        xpool = ctx.enter_context(tc.tile_pool(name="xpool", bufs=1))
        wpool = ctx.enter_context(tc.tile_pool(name="wpool", bufs=2))
        psum = ctx.enter_context(tc.tile_pool(name="psum", bufs=2, space="PSUM"))
        gates = ctx.enter_context(tc.tile_pool(name="gates", bufs=2))
        ew = ctx.enter_context(tc.tile_pool(name="ew", bufs=2))
        const = ctx.enter_context(tc.tile_pool(name="const", bufs=1))

        bias_t = const.tile([128, NA], f32)
        nc.sync.dma_start(out=bias_t[:], in_=bias[:, :])

        for _rep in range(n_rep):
            _body(nc, mdt, f16, f32, AF,
                  xpool, wpool, psum, gates, ew,
                  xT, w, cxT, epcT, ephT, hyT, cyT, bias_t)

    nc.compile()
    return nc


def _body(nc, mdt, f16, f32, AF, xpool, wpool, psum, gates, ew,
          xT, w, cxT, epcT, ephT, hyT, cyT, bias_t):
    # Lead-in: the first weight block rides the sync ring ahead of everything
    # so the PE's first LDWEIGHTS only waits for it plus xk[0]. ht0's other
    # three blocks are interleaved with the odd xk tiles on the scalar ring
    # so ht0 can run kt-major (all 8 chains at once) as the xk tiles land.
    w_t0 = []
    for gate in range(4):
        t = wpool.tile([128, NKT * 128], mdt, tag=f"w{gate}", name=f"w0_{gate}")
        if gate == 0:
            nc.sync.dma_start(out=t[:], in_=w[0, :, :])
        w_t0.append(t)

    # Resident activations: all K tiles of (input;hx)^T for this core's batch
    # shard, [128, BS] each, split across the two HWDGE rings; ht0's weight
    # blocks ride between the first scalar-ring tiles.
    xk = []
    for kt in range(NKT):
        t = xpool.tile([128, BS], mdt, tag=f"xk{kt}", name=f"xk{kt}")
        eng = nc.sync if kt % 2 == 0 else nc.scalar
        eng.dma_start(out=t[:], in_=xT[kt * 128:(kt + 1) * 128, :])
        xk.append(t)
        if kt in (1, 3, 5):
            gate = (kt + 1) // 2
            nc.scalar.dma_start(out=w_t0[gate][:], in_=w[gate, :, :])

    # Elementwise inputs, host-packed to [128, NHT*BS] so each is one DMA.
    # They ride the SWDGE ring behind the early weight blocks (cx is needed
    # first, the noise terms a little later).
    cx_t = ew.tile([128, NHT * BS], f16, tag="cx")
    ec_t = ew.tile([128, NHT * BS], f16, tag="ec")
    eh_t = ew.tile([128, NHT * BS], f16, tag="eh")

    for ht in range(NHT):
        row = slice(ht * 128, (ht + 1) * 128)
        if ht == 0:
            w_t = w_t0
        else:
            # Later h-tiles' weight blocks go on the SWDGE (gpsimd) ring so
            # they are not queued behind the x tiles.
            w_t = []
            for gate in range(4):
                a = ht * 4 + gate
                t = wpool.tile([128, NKT * 128], mdt, tag=f"w{gate}",
                               name=f"w{ht}_{gate}")
                nc.gpsimd.dma_start(out=t[:], in_=w[a, :, :])
                w_t.append(t)
        if ht == 0:
            nc.gpsimd.dma_start(out=cx_t[:], in_=cxT[:, :])
        elif ht == 1:
            nc.gpsimd.dma_start(out=ec_t[:], in_=epcT[:, :])
            nc.gpsimd.dma_start(out=eh_t[:], in_=ephT[:, :])
        gt = [gates.tile([128, BS], f16, tag=f"g{gate}", name=f"gt{ht}_{gate}")
              for gate in range(4)]

        if ht == 0:
            # kt-major: all 8 (chunk, gate) chains accumulate together so each
            # xk tile is consumed 8x as it lands — the PE never outruns the
            # x DMA stream during the fill.
            ps = [[psum.tile([128, 512], f32, tag=f"ps{gate}",
                             name=f"ps0_{c}_{gate}") for gate in range(4)]
                  for c in range(NBC)]
            for kt in range(NKT):
                for gate in range(4):
                    for c in range(NBC):
                        nc.tensor.matmul(
                            ps[c][gate][:],
                            w_t[gate][:, kt * 128:(kt + 1) * 128],
                            xk[kt][:, c * 512:(c + 1) * 512],
                            start=(kt == 0),
                            stop=(kt == NKT - 1),
                        )
            for c in range(NBC):
                for gate in range(4):
                    func = AF.Tanh if gate == 2 else AF.Sigmoid
                    nc.scalar.activation(gt[gate][:, c * 512:(c + 1) * 512],
                                         ps[c][gate][:], func,
                                         bias=bias_t[:, gate:gate + 1])
                _ew_chunk(nc, AF, gt, cx_t, ec_t, eh_t, cyT, hyT, ht, c)
            continue

        for c in range(NBC):
            sl = slice(c * 512, (c + 1) * 512)
            for gate in range(4):
                a = ht * 4 + gate
                ps = psum.tile([128, 512], f32, tag=f"ps{gate}", name=f"ps{ht}_{c}_{gate}")
                for kt in range(NKT):
                    nc.tensor.matmul(
                        ps[:],
                        w_t[gate][:, kt * 128:(kt + 1) * 128],
                        xk[kt][:, sl],
                        start=(kt == 0),
                        stop=(kt == NKT - 1),
                    )
                func = AF.Tanh if gate == 2 else AF.Sigmoid
                nc.scalar.activation(gt[gate][:, sl], ps[:], func,
                                     bias=bias_t[:, a:a + 1])
            _ew_chunk(nc, AF, gt, cx_t, ec_t, eh_t, cyT, hyT, ht, c)


def _ew_chunk(nc, AF, gt, cx_t, ec_t, eh_t, cyT, hyT, ht, c):
    # Elementwise for one 512-wide chunk (all fp16; noise pre-scaled on host).
    row = slice(ht * 128, (ht + 1) * 128)
    sl = slice(c * 512, (c + 1) * 512)
    esl = slice(ht * 1024 + c * 512, ht * 1024 + (c + 1) * 512)
    i_t, f_t, g_t, o_t = (gt[0][:, sl], gt[1][:, sl],
                          gt[2][:, sl], gt[3][:, sl])
    nc.vector.tensor_mul(f_t, f_t, cx_t[:, esl])       # f*cx
    nc.vector.tensor_mul(i_t, i_t, g_t)                # i*g
    nc.vector.tensor_add(f_t, f_t, i_t)
    nc.vector.tensor_add(f_t, f_t, ec_t[:, esl])       # = cy
    nc.scalar.activation(g_t, f_t, AF.Tanh)            # tanh(cy)
    nc.vector.tensor_mul(o_t, o_t, g_t)                # o*tanh(cy)
    nc.vector.tensor_add(o_t, o_t, eh_t[:, esl])       # = hy
    nc.sync.dma_start(out=cyT[row, sl], in_=f_t)
    nc.sync.dma_start(out=hyT[row, sl], in_=o_t)


def _prep_inputs(input, hx, cx, noise_q, noise_e,
                 weight_ih, weight_hh, bias_ih, bias_hh, eps_c, eps_h,
                 mm_np=np.float16):
    f = lambda a: np.ascontiguousarray(np.asarray(a, dtype=np.float32))
    X = np.concatenate([f(input), f(hx)], axis=1)          # [B, K]
    XT = np.ascontiguousarray(X.T.astype(mm_np))            # [K, B]
    W_cat = np.concatenate([f(weight_ih), f(weight_hh)], axis=1).astype(mm_np)  # [G, K]
    bias_full = f(bias_ih) + f(bias_hh)                     # [G]
    cxT = f(cx).T.astype(np.float16)
    epcT = (np.sqrt(np.asarray(noise_e, dtype=np.float32).reshape(-1)[0])
            * f(eps_c)).T.astype(np.float16)
    ephT = (np.sqrt(np.asarray(noise_q, dtype=np.float32).reshape(-1)[0])
            * f(eps_h)).T.astype(np.float16)

    def pack(a):
        # [n*128, cols] -> [128, n*cols] with the n blocks along the free dim
        n = a.shape[0] // 128
        return np.ascontiguousarray(
            a.reshape(n, 128, a.shape[1]).transpose(1, 0, 2).reshape(128, -1))

    # Per gate-shard j: weight blocks in the exact consumption order
    # (a = ht*4 + gate), each pre-transposed to [k_p, kt*128 + g_c] so the
    # per-partition DMA stride is a single contiguous run.
    w_host, bias_host = [], []
    for j in range(P_G):
        blocks, bcols = [], []
        for ht in range(NHT):
            for gate in range(4):
                g0 = gate * H + j * HS + ht * 128
                blk = W_cat[g0:g0 + 128, :]                        # (g_c, k)
                blocks.append(blk.reshape(128, NKT, 128).transpose(2, 1, 0))
                bcols.append(bias_full[g0:g0 + 128])
        w_host.append(np.ascontiguousarray(
            np.stack(blocks).reshape(NA, 128, NKT * 128)))
        bias_host.append(np.ascontiguousarray(np.stack(bcols, axis=1)))

    in_maps = []
    for bi in range(P_B):
        bcol = slice(bi * BS, (bi + 1) * BS)
        for j in range(P_G):
            hrow = slice(j * HS, (j + 1) * HS)
            in_maps.append({
                "xT": np.ascontiguousarray(XT[:, bcol]),
                "w": w_host[j],
                "bias": bias_host[j],
                "cxT": pack(cxT[hrow, bcol]),
                "epcT": pack(epcT[hrow, bcol]),
                "ephT": pack(ephT[hrow, bcol]),
            })
    return in_maps


def _gather(results):
    hyT = np.empty((H, B), dtype=np.float32)
    cyT = np.empty((H, B), dtype=np.float32)
    idx = 0
    for bi in range(P_B):
        bcol = slice(bi * BS, (bi + 1) * BS)
        for j in range(P_G):
            hrow = slice(j * HS, (j + 1) * HS)
            hyT[hrow, bcol] = results[idx]["hyT"].astype(np.float32)
            cyT[hrow, bcol] = results[idx]["cyT"].astype(np.float32)
            idx += 1
    return np.ascontiguousarray(hyT.T), np.ascontiguousarray(cyT.T)


def kernel(**inputs):
    global _LAST
    from concourse.bass_utils import run_bass_kernel_spmd

    in_maps = _prep_inputs(**inputs)
    nc = _build_nc()
    _LAST = run_bass_kernel_spmd(nc, in_maps, list(range(8)), trace=False)
    return _gather(_LAST.results)


# ---------------------------------------------------------------------------
# Timing helper for test.py (not used by the grading path): chains n_iter
# data-dependent NEFF executions inside one jit with device-resident inputs;
# the (t_N - t_1)/(N-1) slope is the per-execution hardware time.
# ---------------------------------------------------------------------------

def benchmark(inputs, n_iter=9, reps=5, n_rep=5):
    in_maps = _prep_inputs(**inputs)
    nc = _build_nc(n_rep=n_rep)
    per_exec_ns, t1_ns, results = _bench_nc(nc, in_maps, n_iter, reps)
    return per_exec_ns / n_rep, t1_ns, _gather(results)


def _bench_nc(nc, in_maps, n_iter=9, reps=5):
    import time
    import jax
    from jax.sharding import Mesh, PartitionSpec, NamedSharding
    from jax.experimental.shard_map import shard_map
    from concourse import bass2jax, mybir
    from concourse.bass2jax import _bass_exec_p

    bass2jax.install_neuronx_cc_hook()
    assert nc.dbg_addr is None
    partition_name = nc.partition_id_tensor.name if nc.partition_id_tensor else None

    in_names, out_names, out_avals, zero_outs = [], [], [], []
    for alloc in nc.m.functions[0].allocations:
        if not isinstance(alloc, mybir.MemoryLocationSet):
            continue
        name = alloc.memorylocations[0].name
        if alloc.kind == "ExternalInput":
            if name != partition_name:
                in_names.append(name)
        elif alloc.kind == "ExternalOutput":
            shape = tuple(alloc.tensor_shape)
            dtype = mybir.dt.np(alloc.dtype)
            out_names.append(name)
            out_avals.append(jax.core.ShapedArray(shape, dtype))
            zero_outs.append(np.zeros(shape, dtype))
    n_params = len(in_names)
    all_in_names = tuple(in_names + out_names
                         + ([partition_name] if partition_name else []))

    def make_body(iters):
        def _body(*args):
            ins = list(args[:n_params])
            outs = tuple(args[n_params:])

            def one(outs):
                pid = [bass2jax.partition_id_tensor()] if partition_name else []
                return tuple(_bass_exec_p.bind(
                    *ins, *outs, *pid,
                    out_avals=tuple(out_avals),
                    in_names=all_in_names,
                    out_names=tuple(out_names),
                    lowering_input_output_aliases=(),
                    sim_require_finite=True,
                    sim_require_nnan=True,
                    nc=nc,
                ))

            return one(outs)
        return _body

    n_cores = 8
    devices = jax.devices()[:n_cores]
    mesh = Mesh(np.asarray(devices), ("core",))
    spec = NamedSharding(mesh, PartitionSpec("core"))
    in_specs = (PartitionSpec("core"),) * (n_params + len(out_names))
    out_specs = (PartitionSpec("core"),) * len(out_names)

    concat_in = [
        np.concatenate([np.asarray(in_maps[c][name]) for c in range(n_cores)], axis=0)
        for name in in_names
    ]
    concat_zeros = [
        np.zeros((n_cores * z.shape[0], *z.shape[1:]), z.dtype) for z in zero_outs
    ]
    dev_args = [jax.device_put(a, spec) for a in concat_in + concat_zeros]
    jax.block_until_ready(dev_args)

    fn = jax.jit(shard_map(make_body(1), mesh=mesh, in_specs=in_specs,
                           out_specs=out_specs, check_rep=False),
                 keep_unused=True)
    out1 = fn(*dev_args)          # compile + warm
    jax.block_until_ready(out1)

    def timed(iters):
        t0 = time.perf_counter()
        out = None
        for _i in range(iters):
            out = fn(*dev_args)   # async dispatches queue in order
        jax.block_until_ready(out)
        return time.perf_counter() - t0

    # Paired 1-iter / n-iter measurements: the slope of each adjacent pair is
    # immune to slow drift in dispatch overhead (axon tunnel congestion);
    # take the minimum slope across pairs.
    slopes = []
    for _ in range(reps):
        t1 = timed(1)
        tn = timed(n_iter)
        slopes.append((tn - t1) / (n_iter - 1))
    per_exec_ns = min(slopes) * 1e9
    t1 = timed(1)
    results = [
        {name: np.asarray(out1[i]).reshape(n_cores, *out_avals[i].shape)[c]
         for i, name in enumerate(out_names)}
        for c in range(n_cores)
    ]
    return per_exec_ns, t1 * 1e9, results
